# revision 12
# baseline (speedup 1.0000x reference)
"""GAT layer (nn_ManualGATLayer) Bass/Tile kernel for 8 Trainium2 cores.

Math (per head h, batch b):
    Wh   = h_b @ W_h.T                          [N, D]
    si   = Wh @ a1,  sj = Wh @ a2               [N]
    e_ij = leaky(si_i + sj_j), masked by adj, softmax over j, out = alpha @ Wh

Key identities:
  1) leaky(x) = max(x, 0.2x) and exp is monotone, so
         exp(leaky(si_i + sj_j)) = max(Ei*Ej, Fi*Fj)
     with Ei = exp(si), Ej = exp(sj), Fi = exp(0.2 si), Fj = exp(0.2 sj).
  2) Softmax over j is invariant to any per-i row scale, so we compute
         P'_ij = P_ij / Ei = max(Hi_i * Fj_j, Ej_j),   Hi = exp(-0.8 si)
     In the [j(128-part) x i(free)] tile layout, Hi is the broadcast
     tensor and Ej/Fj are per-partition scalars, so the WHOLE N^2 score
     needs ONE two-op tensor_scalar:  (Hi_bc mult Fj[p]) max Ej[p]
     running in 4x bf16 mode (~594ns/tile vs 2315ns for the 3-op split).

The adjacency mask is folded into the DMA load: adjT' = {0, -57344}
stored fp8e5m2 is added onto the score tile via SWDGE accum_op=add;
-57344 swamps any score (<~100) and a relu restores exact zeros.

Consume is "flipped": the P' tile is the matmul STATIONARY operand
(16 blocks of [128j x 128i] per tile) and the moving operand is
wh_ext = [Wh_tile | 4.0] (129 cols), so psum[i, 0:128] accumulates
out-numerator^T' and psum[i, 128] accumulates 4*r'_i: the rowsum rides
along for +1 column instead of a second full matmul pass. The host
divides by column 128 (softmax denominator; the 4.0 folds the H-head
mean), which also cancels the 1/Ei row scale exactly.

Sharding: one (h, b) pair per core (H*B = 8 = n_cores); host sums heads.
"""

import numpy as np
import ml_dtypes

BF16 = ml_dtypes.bfloat16
NEG_SLOPE = 0.2
MASK_VAL = -57344.0

# Problem sizes (hardcoded per contest contract).
B, N, IN, D, H = 2, 4096, 256, 128, 4
N_CORES = 8

_cache = {}


def _build(n=N, n_in=IN, d=D, num_devices=N_CORES, repeat=1, skip=(),
           relu_act_every=4, score_2op=True, jt_group=2):
    # skip: subset of {"dve", "dma", "relu", "mm"} for timing-attribution
    # variants (numerically wrong where used).
    # relu_act_every: every k-th tile's relu runs on ACT instead of DVE
    # (0 = never, 1 = always).
    import concourse.bacc as bacc
    import concourse.tile as tile
    from concourse import mybir

    f32 = mybir.dt.float32
    bf16 = mybir.dt.bfloat16
    AF = mybir.ActivationFunctionType
    ALU = mybir.AluOpType

    n_jt = n // 128          # j tiles of 128
    ih_n = 2                 # i halves
    iw = n // ih_n           # i width per half (2048)
    n_ib = iw // 128         # i blocks per half (16) -- two per PSUM bank
    n_kt = n_in // 128       # contraction tiles for Wh

    nc = bacc.Bacc(
        "TRN2",
        target_bir_lowering=False,
        debug=False,
        num_devices=num_devices,
    )

    hb = nc.dram_tensor("hb", [n, n_in], bf16, kind="ExternalInput")
    wt = nc.dram_tensor("wt", [n_in, d], bf16, kind="ExternalInput")
    a12 = nc.dram_tensor("a12", [d, 2], bf16, kind="ExternalInput")
    adjt = nc.dram_tensor("adjt", [n, n], mybir.dt.float8e5,
                          kind="ExternalInput")
    # out rows: [i, 0:128] = unnormalized out^T', [i, 128] = 4*r'_i
    outR = nc.dram_tensor("outR", [n, d + 1], f32, kind="ExternalOutput")

    with tile.TileContext(nc) as tc:
        with tc.tile_pool(name="const", bufs=1) as const:
            # --- constants and persistent tiles ---
            wt_sb = const.tile([128, n_kt, d], bf16)
            nc.sync.dma_start(
                out=wt_sb, in_=wt[:].rearrange("(k p) d -> p k d", p=128)
            )
            a12_sb = const.tile([d, 2], bf16)
            nc.sync.dma_start(out=a12_sb, in_=a12[:])
            ones1 = const.tile([1, 128], f32)
            nc.vector.memset(ones1, 1.0)

            # hT[k] = h[:, k*128:(k+1)*128].T  via DMA xbar transpose
            hT = const.tile([128, n_kt, n], bf16)
            for k in range(n_kt):
                nc.sync.dma_start(
                    out=hT[:, k, :],
                    in_=hb[:, k * 128 : (k + 1) * 128],
                    transpose=True,
                )

            # --- wh_ext tiles [n-tile, 129] (bf16): [Wh | 4.0] ---
            wh_ext = const.tile([128, n_jt, d + 1], bf16)
            nc.vector.memset(wh_ext, float(H))  # col d = 4.0 (head mean)
            with tc.tile_pool(name="ps_wh", bufs=2, space="PSUM") as ps_wh:
                for g in range(n_jt // 4):
                    wh_ps = ps_wh.tile([128, 4, d], f32, tag="wh_ps")
                    for q in range(4):
                        nt = g * 4 + q
                        for k in range(n_kt):
                            nc.tensor.matmul(
                                wh_ps[:, q, :],
                                hT[:, k, nt * 128 : (nt + 1) * 128],
                                wt_sb[:, k, :],
                                start=(k == 0),
                                stop=(k == n_kt - 1),
                            )
                    for q in range(4):
                        nc.vector.tensor_copy(
                            wh_ext[:, g * 4 + q, 0:d], wh_ps[:, q, :]
                        )

            # --- WhT [d, n] (bf16) ---
            whT_sb = const.tile([128, n], bf16)
            with tc.tile_pool(name="ps_whT", bufs=1, space="PSUM") as ps_whT:
                whT_ps = ps_whT.tile([128, n], f32)
                for c in range(n // 512):
                    for k in range(n_kt):
                        nc.tensor.matmul(
                            whT_ps[:, c * 512 : (c + 1) * 512],
                            wt_sb[:, k, :],
                            hT[:, k, c * 512 : (c + 1) * 512],
                            start=(k == 0),
                            stop=(k == n_kt - 1),
                        )
                nc.vector.tensor_copy(whT_sb, whT_ps)

            # --- si/sj row vectors [2, n] f32 ---
            s_sb = const.tile([2, n], f32)
            with tc.tile_pool(name="ps_s", bufs=1, space="PSUM") as ps_s:
                s_ps = ps_s.tile([2, n], f32)
                for c in range(n // 512):
                    nc.tensor.matmul(
                        s_ps[:, c * 512 : (c + 1) * 512],
                        a12_sb,
                        whT_sb[:, c * 512 : (c + 1) * 512],
                        start=True,
                        stop=True,
                    )
                nc.vector.tensor_copy(s_sb, s_ps)

            # --- Hi broadcast tile [128, n] bf16: exp(-0.8 si) ---
            Hi_bc = const.tile([128, n], bf16)
            with tc.tile_pool(name="ps_sib", bufs=1, space="PSUM") as ps_sib:
                sib_ps = ps_sib.tile([128, n], f32)
                for c in range(n // 512):
                    nc.tensor.matmul(
                        sib_ps[:, c * 512 : (c + 1) * 512],
                        ones1,
                        s_sb[0:1, c * 512 : (c + 1) * 512],
                        start=True,
                        stop=True,
                    )
                nc.scalar.activation(Hi_bc, sib_ps, AF.Exp, scale=-0.8)

            # --- Ej/Fj per-partition columns [128, n_jt] f32 ---
            Ej_cols = const.tile([128, n_jt], f32)
            Fj_cols = const.tile([128, n_jt], f32)
            with tc.tile_pool(name="ps_sj", bufs=1, space="PSUM") as ps_sj:
                sj_ps = ps_sj.tile([128, n_jt], f32)
                for t in range(n_jt):
                    nc.tensor.matmul(
                        sj_ps[:, t : t + 1],
                        whT_sb[:, t * 128 : (t + 1) * 128],
                        a12_sb[:, 1:2],
                        start=True,
                        stop=True,
                    )
                nc.scalar.activation(Ej_cols, sj_ps, AF.Exp)
                nc.scalar.activation(Fj_cols, sj_ps, AF.Exp, scale=NEG_SLOPE)

            # --- main attention loop ---
            with (
                tc.tile_pool(name="work", bufs=8) as work,
                tc.tile_pool(name="fin", bufs=2) as fin,
                tc.tile_pool(name="ps_main", bufs=1, space="PSUM") as ps_main,
            ):
                for ih in [x for x in range(ih_n)] * repeat:
                    i0 = ih * iw
                    # 16 psum blocks of [128 i, 129(pad 256)] f32, two per
                    # 2KB bank.  start=True clears has_written for the WHOLE
                    # bank, so only the even block of each bank pair issues
                    # it; the odd block's first matmul relies on
                    # "overwrite where has_written is unset" (its bits were
                    # cleared by the even neighbor's start, which the issue
                    # order guarantees happens first).
                    out_ps = ps_main.tile([128, n_ib, 256], f32, tag="out_ps")
                    for jg in range(n_jt // jt_group):
                        jt0 = jg * jt_group
                        m = work.tile([128, jt_group, iw], bf16, tag="m")
                        if "dve" not in skip:
                            # P'/relu pre-mask: max(Hi_i * Fj_j, Ej_j)
                            for q in range(jt_group):
                                nc.vector.tensor_scalar(
                                    m[:, q, :],
                                    Hi_bc[:, i0 : i0 + iw],
                                    Fj_cols[:, jt0 + q : jt0 + q + 1],
                                    Ej_cols[:, jt0 + q : jt0 + q + 1],
                                    ALU.mult,
                                    ALU.max,
                                )
                        else:
                            nc.vector.memset(m, 1.0)
                        if "dma" not in skip:
                            # fold adjacency mask in during the load:
                            #   m += adjT' ({0, -57344}), then P' = relu(m).
                            # One batched DMA per jt_group j-tiles.
                            nc.gpsimd.dma_start(
                                out=m,
                                in_=adjt[jt0 * 128 : (jt0 + jt_group) * 128,
                                         i0 : i0 + iw].rearrange(
                                    "(g p) i -> p g i", p=128
                                ),
                                accum_op=ALU.add,
                            )
                        for q in range(jt_group):
                            jt = jt0 + q
                            if "relu" not in skip:
                                p = work.tile([128, iw], bf16, tag="p")
                                if relu_act_every and jt % relu_act_every == 0:
                                    nc.scalar.activation(p, m[:, q, :], AF.Relu)
                                else:
                                    nc.vector.tensor_scalar_max(
                                        p, m[:, q, :], 0.0
                                    )
                            else:
                                p = m[:, q, :]
                            if "mm" in skip:
                                continue
                            for bi in range(n_ib):
                                nc.tensor.matmul(
                                    out_ps[:, bi, 0 : d + 1],
                                    p[:, bi * 128 : (bi + 1) * 128],
                                    wh_ext[:, jt, :],
                                    start=(jt == 0 and bi % 2 == 0),
                                    stop=(jt == n_jt - 1),
                                    skip_group_check=(bi % 2 == 1),
                                )
                    # drain: one ACT copy for all 16 blocks, then DMA out
                    out_sb = fin.tile([128, n_ib, d + 1], f32, tag="out_sb")
                    if "mm" in skip:
                        nc.vector.memset(out_sb, 1.0)
                    else:
                        nc.scalar.activation(
                            out_sb, out_ps[:, :, 0 : d + 1], AF.Copy
                        )
                    nc.sync.dma_start(
                        out=outR[i0 : i0 + iw, :].rearrange(
                            "(b p) c -> p b c", p=128
                        ),
                        in_=out_sb,
                    )

    nc.compile()
    return nc


def _prep_inputs(h, adj, W, a):
    """Host-side shard/layout prep. Returns list of 8 per-core input dicts."""
    h_bf = np.asarray(h).astype(BF16)
    adjt_big = np.where(np.asarray(adj).T != 0, 0.0, MASK_VAL).astype(
        ml_dtypes.float8_e5m2
    )
    adjt_big = np.ascontiguousarray(adjt_big)
    W = np.asarray(W)
    a = np.asarray(a)
    in_maps = []
    for c in range(N_CORES):
        hd, b = divmod(c, B)
        wt = np.ascontiguousarray(W[hd].T).astype(BF16)          # [IN, D]
        a12 = np.stack([a[hd, :D], a[hd, D:]], axis=1).astype(BF16)  # [D, 2]
        in_maps.append(
            {"hb": np.ascontiguousarray(h_bf[b]), "wt": wt, "a12": a12,
             "adjt": adjt_big}
        )
    return in_maps


def kernel(h, adj, W, a):
    from concourse.bass_utils import run_bass_kernel_spmd

    if "nc" not in _cache:
        _cache["nc"] = _build()
    nc = _cache["nc"]

    in_maps = _prep_inputs(h, adj, W, a)
    res = run_bass_kernel_spmd(nc, in_maps, core_ids=list(range(N_CORES)))
    outs = [r["outR"] for r in res.results]  # each [N, D+1] f32

    out = np.zeros((B, N, D), dtype=np.float32)
    for c in range(N_CORES):
        hd, b = divmod(c, B)
        o = outs[c]
        r = o[:, D:]
        out[b] += np.divide(o[:, :D], r, out=np.zeros((N, D), np.float32),
                            where=r != 0)
    return out


# revision 19
# speedup vs baseline: 1.2849x; 1.2849x over previous
"""GAT layer (nn_ManualGATLayer) Bass/Tile kernel for 8 Trainium2 cores.

Math (per head h, batch b):
    Wh   = h_b @ W_h.T                          [N, D]
    si   = Wh @ a1,  sj = Wh @ a2               [N]
    e_ij = leaky(si_i + sj_j), masked by adj, softmax over j, out = alpha @ Wh

Key identities:
  1) leaky(x) = max(x, 0.2x) and exp is monotone, so
         exp(leaky(si_i + sj_j)) = max(Ei*Ej, Fi*Fj)
     with Ei = exp(si), Ej = exp(sj), Fi = exp(0.2 si), Fj = exp(0.2 sj).
  2) Softmax over j is invariant to any per-i row scale, so we compute
         P'_ij = P_ij / Ei = max(Hi_i * Fj_j, Ej_j),   Hi = exp(-0.8 si)
     In the [j(128-part) x i(free)] tile layout, Hi is the broadcast
     tensor and Ej/Fj are per-partition scalars, so the WHOLE N^2 score
     needs ONE two-op tensor_scalar:  (Hi_bc mult Fj[p]) max Ej[p]
     running in 4x bf16 mode (~594ns/tile vs 2315ns for the 3-op split).

The adjacency mask is folded into the DMA load: adjT' = {0, -57344}
stored fp8e5m2 is added onto the score tile via SWDGE accum_op=add;
-57344 swamps any score (<~100) and a relu restores exact zeros.

Consume is "flipped": the P' tile is the matmul STATIONARY operand
(16 blocks of [128j x 128i] per tile) and the moving operand is
wh_ext = [Wh_tile | 4.0] (129 cols), so psum[i, 0:128] accumulates
out-numerator^T' and psum[i, 128] accumulates 4*r'_i: the rowsum rides
along for +1 column instead of a second full matmul pass. The host
divides by column 128 (softmax denominator; the 4.0 folds the H-head
mean), which also cancels the 1/Ei row scale exactly.

Sharding: one (h, b) pair per core (H*B = 8 = n_cores); host sums heads.
"""

import numpy as np
import ml_dtypes

BF16 = ml_dtypes.bfloat16
NEG_SLOPE = 0.2
MASK_VAL = -57344.0

# Problem sizes (hardcoded per contest contract).
B, N, IN, D, H = 2, 4096, 256, 128, 4
N_CORES = 8

_cache = {}


def _build(n=N, n_in=IN, d=D, num_devices=N_CORES, repeat=1, skip=(),
           relu_act_every=1, score_2op=True, jt_group=2, accum_every=6):
    # skip: subset of {"dve", "dma", "relu", "mm"} for timing-attribution
    # variants (numerically wrong where used).
    # relu_act_every: every k-th tile's relu runs on ACT instead of DVE
    # (0 = never, 1 = always).
    import concourse.bacc as bacc
    import concourse.tile as tile
    from concourse import mybir

    f32 = mybir.dt.float32
    bf16 = mybir.dt.bfloat16
    AF = mybir.ActivationFunctionType
    ALU = mybir.AluOpType

    n_jt = n // 128          # j tiles of 128
    ih_n = 2                 # i halves
    iw = n // ih_n           # i width per half (2048)
    n_ib = iw // 128         # i blocks per half (16) -- two per PSUM bank
    n_kt = n_in // 128       # contraction tiles for Wh

    nc = bacc.Bacc(
        "TRN2",
        target_bir_lowering=False,
        debug=False,
        num_devices=num_devices,
    )

    hb = nc.dram_tensor("hb", [n, n_in], bf16, kind="ExternalInput")
    wt = nc.dram_tensor("wt", [n_in, d], bf16, kind="ExternalInput")
    a12 = nc.dram_tensor("a12", [d, 2], bf16, kind="ExternalInput")
    adjt = nc.dram_tensor("adjt", [n, n], mybir.dt.float8e5,
                          kind="ExternalInput")
    adjm = nc.dram_tensor("adjm", [n, n], bf16, kind="ExternalInput")
    # out rows: [i, 0:128] = unnormalized out^T', [i, 128] = 4*r'_i
    outR = nc.dram_tensor("outR", [n, d + 1], f32, kind="ExternalOutput")

    with tile.TileContext(nc) as tc:
        with tc.tile_pool(name="const", bufs=1) as const:
            # --- constants and persistent tiles ---
            wt_sb = const.tile([128, n_kt, d], bf16)
            nc.sync.dma_start(
                out=wt_sb, in_=wt[:].rearrange("(k p) d -> p k d", p=128)
            )
            a12_sb = const.tile([d, 2], bf16)
            nc.sync.dma_start(out=a12_sb, in_=a12[:])
            ones1 = const.tile([1, 128], f32)
            nc.vector.memset(ones1, 1.0)

            # hT[k] = h[:, k*128:(k+1)*128].T  via DMA xbar transpose
            hT = const.tile([128, n_kt, n], bf16)
            for k in range(n_kt):
                nc.sync.dma_start(
                    out=hT[:, k, :],
                    in_=hb[:, k * 128 : (k + 1) * 128],
                    transpose=True,
                )

            # --- wh_ext tiles [n-tile, 129] (bf16): [Wh | 4.0] ---
            wh_ext = const.tile([128, n_jt, d + 1], bf16)
            nc.vector.memset(wh_ext, float(H))  # col d = 4.0 (head mean)
            with tc.tile_pool(name="ps_wh", bufs=2, space="PSUM") as ps_wh:
                for g in range(n_jt // 4):
                    wh_ps = ps_wh.tile([128, 4, d], f32, tag="wh_ps")
                    for q in range(4):
                        nt = g * 4 + q
                        for k in range(n_kt):
                            nc.tensor.matmul(
                                wh_ps[:, q, :],
                                hT[:, k, nt * 128 : (nt + 1) * 128],
                                wt_sb[:, k, :],
                                start=(k == 0),
                                stop=(k == n_kt - 1),
                            )
                    for q in range(4):
                        nc.vector.tensor_copy(
                            wh_ext[:, g * 4 + q, 0:d], wh_ps[:, q, :]
                        )

            # --- WhT [d, n] (bf16) ---
            whT_sb = const.tile([128, n], bf16)
            with tc.tile_pool(name="ps_whT", bufs=1, space="PSUM") as ps_whT:
                whT_ps = ps_whT.tile([128, n], f32)
                for c in range(n // 512):
                    for k in range(n_kt):
                        nc.tensor.matmul(
                            whT_ps[:, c * 512 : (c + 1) * 512],
                            wt_sb[:, k, :],
                            hT[:, k, c * 512 : (c + 1) * 512],
                            start=(k == 0),
                            stop=(k == n_kt - 1),
                        )
                nc.vector.tensor_copy(whT_sb, whT_ps)

            # --- si/sj row vectors [2, n] f32 ---
            s_sb = const.tile([2, n], f32)
            with tc.tile_pool(name="ps_s", bufs=1, space="PSUM") as ps_s:
                s_ps = ps_s.tile([2, n], f32)
                for c in range(n // 512):
                    nc.tensor.matmul(
                        s_ps[:, c * 512 : (c + 1) * 512],
                        a12_sb,
                        whT_sb[:, c * 512 : (c + 1) * 512],
                        start=True,
                        stop=True,
                    )
                nc.vector.tensor_copy(s_sb, s_ps)

            # --- Hi broadcast tile [128, n] bf16: exp(-0.8 si) ---
            Hi_bc = const.tile([128, n], bf16)
            with tc.tile_pool(name="ps_sib", bufs=1, space="PSUM") as ps_sib:
                sib_ps = ps_sib.tile([128, n], f32)
                for c in range(n // 512):
                    nc.tensor.matmul(
                        sib_ps[:, c * 512 : (c + 1) * 512],
                        ones1,
                        s_sb[0:1, c * 512 : (c + 1) * 512],
                        start=True,
                        stop=True,
                    )
                nc.scalar.activation(Hi_bc, sib_ps, AF.Exp, scale=-0.8)

            # --- Ej/Fj per-partition columns [128, n_jt] f32 ---
            Ej_cols = const.tile([128, n_jt], f32)
            Fj_cols = const.tile([128, n_jt], f32)
            with tc.tile_pool(name="ps_sj", bufs=1, space="PSUM") as ps_sj:
                sj_ps = ps_sj.tile([128, n_jt], f32)
                for t in range(n_jt):
                    nc.tensor.matmul(
                        sj_ps[:, t : t + 1],
                        whT_sb[:, t * 128 : (t + 1) * 128],
                        a12_sb[:, 1:2],
                        start=True,
                        stop=True,
                    )
                nc.scalar.activation(Ej_cols, sj_ps, AF.Exp)
                nc.scalar.activation(Fj_cols, sj_ps, AF.Exp, scale=NEG_SLOPE)

            # --- main attention loop ---
            work_bufs = max(2, 16 // jt_group)
            with (
                tc.tile_pool(name="work", bufs=work_bufs) as work,
                tc.tile_pool(name="fin", bufs=2) as fin,
                tc.tile_pool(name="ps_main", bufs=1, space="PSUM") as ps_main,
            ):
                for ih in [x for x in range(ih_n)] * repeat:
                    i0 = ih * iw
                    # 16 psum blocks of [128 i, 129(pad 256)] f32, two per
                    # 2KB bank.  start=True clears has_written for the WHOLE
                    # bank, so only the even block of each bank pair issues
                    # it; the odd block's first matmul relies on
                    # "overwrite where has_written is unset" (its bits were
                    # cleared by the even neighbor's start, which the issue
                    # order guarantees happens first).
                    out_ps = ps_main.tile([128, n_ib, 256], f32, tag="out_ps")
                    for jt in range(n_jt):
                        use_accum = (
                            accum_every > 0 and jt % accum_every == 0
                        )
                        m = work.tile([128, iw], bf16, tag="m")
                        if not use_accum and "dma" not in skip:
                            # {0,1} bf16 adjacency slab, plain HWDGE load
                            a_t = work.tile([128, iw], bf16, tag="a_t")
                            nc.sync.dma_start(
                                out=a_t,
                                in_=adjm[jt * 128 : (jt + 1) * 128,
                                         i0 : i0 + iw],
                            )
                        if "dve" not in skip:
                            # P'/pre-mask: max(Hi_i * Fj_j, Ej_j)
                            nc.vector.tensor_scalar(
                                m,
                                Hi_bc[:, i0 : i0 + iw],
                                Fj_cols[:, jt : jt + 1],
                                Ej_cols[:, jt : jt + 1],
                                ALU.mult,
                                ALU.max,
                            )
                        else:
                            nc.vector.memset(m, 1.0)
                        if use_accum:
                            # fold mask in during the load (SWDGE CCE add,
                            # ~104GB/s): m += {0, -57344}, relu restores 0s.
                            if "dma" not in skip:
                                nc.gpsimd.dma_start(
                                    out=m,
                                    in_=adjt[jt * 128 : (jt + 1) * 128,
                                             i0 : i0 + iw],
                                    accum_op=ALU.add,
                                )
                            p = m
                            if "relu" not in skip:
                                if relu_act_every:
                                    nc.scalar.activation(p, p, AF.Relu)
                                else:
                                    nc.vector.tensor_scalar_max(p, p, 0.0)
                        else:
                            # multiplicative mask: no relu needed
                            p = work.tile([128, iw], bf16, tag="p")
                            if "dma" not in skip and "relu" not in skip:
                                nc.vector.tensor_mul(p, m, a_t)
                            else:
                                p = m
                        if "mm" in skip:
                            continue
                        for bi in range(n_ib):
                            nc.tensor.matmul(
                                out_ps[:, bi, 0 : d + 1],
                                p[:, bi * 128 : (bi + 1) * 128],
                                wh_ext[:, jt, :],
                                start=(jt == 0 and bi % 2 == 0),
                                stop=(jt == n_jt - 1),
                                skip_group_check=(bi % 2 == 1),
                            )
                    # drain: one ACT copy for all 16 blocks, then DMA out
                    out_sb = fin.tile([128, n_ib, d + 1], f32, tag="out_sb")
                    if "mm" in skip:
                        nc.vector.memset(out_sb, 1.0)
                    else:
                        nc.scalar.activation(
                            out_sb, out_ps[:, :, 0 : d + 1], AF.Copy
                        )
                    nc.sync.dma_start(
                        out=outR[i0 : i0 + iw, :].rearrange(
                            "(b p) c -> p b c", p=128
                        ),
                        in_=out_sb,
                    )

    nc.compile()
    return nc


def _prep_inputs(h, adj, W, a):
    """Host-side shard/layout prep. Returns list of 8 per-core input dicts."""
    h_bf = np.asarray(h).astype(BF16)
    adjT = np.asarray(adj).T != 0
    adjt_big = np.ascontiguousarray(
        np.where(adjT, 0.0, MASK_VAL).astype(ml_dtypes.float8_e5m2)
    )
    adjm_big = np.ascontiguousarray(adjT.astype(BF16))
    W = np.asarray(W)
    a = np.asarray(a)
    in_maps = []
    for c in range(N_CORES):
        hd, b = divmod(c, B)
        wt = np.ascontiguousarray(W[hd].T).astype(BF16)          # [IN, D]
        a12 = np.stack([a[hd, :D], a[hd, D:]], axis=1).astype(BF16)  # [D, 2]
        in_maps.append(
            {"hb": np.ascontiguousarray(h_bf[b]), "wt": wt, "a12": a12,
             "adjt": adjt_big, "adjm": adjm_big}
        )
    return in_maps


def kernel(h, adj, W, a):
    from concourse.bass_utils import run_bass_kernel_spmd

    if "nc" not in _cache:
        _cache["nc"] = _build()
    nc = _cache["nc"]

    in_maps = _prep_inputs(h, adj, W, a)
    res = run_bass_kernel_spmd(nc, in_maps, core_ids=list(range(N_CORES)))
    outs = [r["outR"] for r in res.results]  # each [N, D+1] f32

    out = np.zeros((B, N, D), dtype=np.float32)
    for c in range(N_CORES):
        hd, b = divmod(c, B)
        o = outs[c]
        r = o[:, D:]
        out[b] += np.divide(o[:, :D], r, out=np.zeros((N, D), np.float32),
                            where=r != 0)
    return out


# revision 31
# speedup vs baseline: 2.6187x; 2.0381x over previous
"""GAT layer (nn_ManualGATLayer) Bass/Tile kernel for 8 Trainium2 cores.

Math (per head h, batch b):
    Wh   = h_b @ W_h.T                          [N, D]
    si   = Wh @ a1,  sj = Wh @ a2               [N]
    e_ij = leaky(si_i + sj_j), masked by adj, softmax over j, out = alpha @ Wh

Key identities:
  1) leaky(x) = max(x, 0.2x) and exp is monotone, so
         exp(leaky(si_i + sj_j)) = max(Ei*Ej, Fi*Fj).
  2) Softmax over j is invariant to any per-i row scale, so we compute
         P'_ij = P_ij / Ei = max(Hi_i * Fj_j, Ej_j),   Hi = exp(-0.8 si)
     In the [j(128-part) x i(free)] tile layout Hi is the broadcast
     tensor and Ej/Fj are per-partition scalars, so the WHOLE N^2 score
     needs ONE two-op tensor_scalar in 4x bf16 mode.

Sharding (G=2/S=4): core c = (g, q) with g = c//4, q = c%4 handles the
4 pairs {(2g,0),(2g,1),(2g+1,0),(2g+1,1)} (head, batch) restricted to
query nodes i in [1024*q, 1024*(q+1)).  The payoff: the {0,1} bf16
adjacency slice [4096 j, 1024 i] (8MB) is DMA'd into SBUF ONCE and
reused by all 4 pairs' mask-multiplies, cutting per-core mask DMA ~4x
versus one-pair-per-core (the mask was the measured bottleneck: SWDGE
CCE-add runs ~104GB/s and per-tile bf16 loads cost 2B/elem).

The mask is applied by a hybrid, balanced so neither DMA nor DVE binds:
  - most tiles: one DVE tensor_tensor multiply (2x mode) against the
    SBUF-resident {0,1} slab -- no relu needed;
  - every accum_every-th jt: adjT' = {0, -57344} fp8e5m2 added onto the
    score tile during its load via SWDGE accum_op=add, then an ACT relu
    restores exact zeros (moves that tile's mask work off DVE).

Consume is "flipped": the P' tile is the matmul STATIONARY operand
(8 blocks of [128j x 128i]) and the moving operand is [Wh | 4.0]
(129 cols): psum[i, 0:128] accumulates the numerator and psum[i, 128]
accumulates 4*r'_i -- the softmax denominator rides along as one extra
column.  PSUM per pair is 8 blocks x 1KB = 4 banks, double-buffered
across pairs.  The host divides by column 128 (cancels the 1/Ei row
scale exactly; the 4.0 folds the H-head mean) and sums head cores.
"""

import numpy as np
import ml_dtypes

BF16 = ml_dtypes.bfloat16
NEG_SLOPE = 0.2
MASK_VAL = -57344.0

# Problem sizes (hardcoded per contest contract).
B, N, IN, D, H = 2, 4096, 256, 128, 4
N_CORES = 8
N_GRP = 2     # pair groups (heads 2g, 2g+1 x both batches)
N_Q = 4       # i-quarters
IW = N // N_Q # 1024

_cache = {}


def _build(n=N, n_in=IN, d=D, num_devices=N_CORES, repeat=1, skip=(),
           accum_every=3, accum_sbuf=False):
    # skip: subset of {"dve", "dma", "relu", "mm"} for timing-attribution
    # variants (numerically wrong where used).
    import concourse.bacc as bacc
    import concourse.tile as tile
    from concourse import mybir

    f32 = mybir.dt.float32
    bf16 = mybir.dt.bfloat16
    AF = mybir.ActivationFunctionType
    ALU = mybir.AluOpType

    n_jt = n // 128          # j tiles of 128
    iw = IW                  # i width per core (1024)
    n_ib = iw // 128         # i blocks (8) -- 2 per PSUM bank
    n_kt = n_in // 128       # contraction tiles for Wh
    n_pair = 4

    nc = bacc.Bacc(
        "TRN2",
        target_bir_lowering=False,
        debug=False,
        num_devices=num_devices,
    )

    hb = nc.dram_tensor("hb", [B, n, n_in], bf16, kind="ExternalInput")
    hq = nc.dram_tensor("hq", [B, IW, n_in], bf16, kind="ExternalInput")
    wt2 = nc.dram_tensor("wt2", [2, n_in, d], bf16, kind="ExternalInput")
    a12 = nc.dram_tensor("a12", [2, d, 2], bf16, kind="ExternalInput")
    adjt = nc.dram_tensor("adjt", [n, iw], mybir.dt.float8e5,
                          kind="ExternalInput")
    adjm = nc.dram_tensor("adjm", [n, iw], bf16, kind="ExternalInput")
    # per pair: [i, 0:128] = unnormalized out^T', [i, 128] = 4*r'_i
    outR = nc.dram_tensor("outR", [n_pair, iw, d + 1], f32,
                          kind="ExternalOutput")

    with tile.TileContext(nc) as tc:
        with tc.tile_pool(name="const", bufs=1) as const:
            # --- persistent tiles ---
            # {0,1} bf16 adjacency slab, loaded once, reused by all pairs
            adjm_sb = const.tile([128, n_jt, iw], bf16)
            nc.sync.dma_start(
                out=adjm_sb, in_=adjm[:].rearrange("(t p) i -> p t i", p=128)
            )
            if accum_sbuf:
                adjt_sb = const.tile([128, n_jt, iw], mybir.dt.float8e5)
                nc.sync.dma_start(
                    out=adjt_sb,
                    in_=adjt[:].rearrange("(t p) i -> p t i", p=128),
                )
            a12_sb = const.tile([d, 2, 2], bf16)
            nc.sync.dma_start(
                out=a12_sb, in_=a12[:].rearrange("h d c -> d h c")
            )
            ones1 = const.tile([1, 128], f32)
            nc.vector.memset(ones1, 1.0)

            # per-pair score vectors (pair = (head hp, batch bp))
            wh_ext = const.tile([128, n_pair, n_jt, d + 1], bf16)
            nc.vector.memset(wh_ext, float(H))  # col d = 4.0 (head mean)
            Hi_bc = const.tile([128, n_pair, iw], bf16)
            Ej_cols = const.tile([128, n_pair, n_jt], f32)
            Fj_cols = const.tile([128, n_pair, n_jt], f32)

            with tc.tile_pool(name="prolog", bufs=1) as prolog:
                wt_sb = prolog.tile([128, 2, n_kt, d], bf16)
                nc.sync.dma_start(
                    out=wt_sb,
                    in_=wt2[:].rearrange("h (k p) d -> p h k d", p=128),
                )
                hT = prolog.tile([128, B, n_kt, n], bf16)
                hqT = prolog.tile([128, B, n_kt, iw], bf16)
                for bb in range(B):
                    for k in range(n_kt):
                        nc.sync.dma_start(
                            out=hT[:, bb, k, :],
                            in_=hb[bb, :, k * 128 : (k + 1) * 128],
                            transpose=True,
                        )
                        nc.sync.dma_start(
                            out=hqT[:, bb, k, :],
                            in_=hq[bb, :, k * 128 : (k + 1) * 128],
                            transpose=True,
                        )
                for pp in range(n_pair):
                    hp, bp = divmod(pp, B)
                    whT_sb = prolog.tile([128, n], bf16, tag="whT")
                    # Wh tiles [n-tile, d] into wh_ext cols 0:d
                    with tc.tile_pool(name="ps_wh", bufs=2,
                                      space="PSUM") as ps_wh:
                        for g4 in range(n_jt // 4):
                            wh_ps = ps_wh.tile([128, 4, d], f32, tag="wh")
                            for q4 in range(4):
                                nt = g4 * 4 + q4
                                for k in range(n_kt):
                                    nc.tensor.matmul(
                                        wh_ps[:, q4, :],
                                        hT[:, bp, k,
                                           nt * 128 : (nt + 1) * 128],
                                        wt_sb[:, hp, k, :],
                                        start=(k == 0),
                                        stop=(k == n_kt - 1),
                                    )
                            for q4 in range(4):
                                nc.vector.tensor_copy(
                                    wh_ext[:, pp, g4 * 4 + q4, 0:d],
                                    wh_ps[:, q4, :],
                                )
                    # WhT [d, n]
                    with tc.tile_pool(name="ps_whT", bufs=1,
                                      space="PSUM") as ps_whT:
                        whT_ps = ps_whT.tile([128, n], f32)
                        for c in range(n // 512):
                            for k in range(n_kt):
                                nc.tensor.matmul(
                                    whT_ps[:, c * 512 : (c + 1) * 512],
                                    wt_sb[:, hp, k, :],
                                    hT[:, bp, k, c * 512 : (c + 1) * 512],
                                    start=(k == 0),
                                    stop=(k == n_kt - 1),
                                )
                        nc.vector.tensor_copy(whT_sb, whT_ps)
                    # WhqT [d, iw] for this core's i-quarter
                    whqT_sb = prolog.tile([128, iw], bf16, tag="whqT")
                    with tc.tile_pool(name="ps_wq", bufs=1,
                                      space="PSUM") as ps_wq:
                        wq_ps = ps_wq.tile([128, iw], f32)
                        for c in range(iw // 512):
                            for k in range(n_kt):
                                nc.tensor.matmul(
                                    wq_ps[:, c * 512 : (c + 1) * 512],
                                    wt_sb[:, hp, k, :],
                                    hqT[:, bp, k, c * 512 : (c + 1) * 512],
                                    start=(k == 0),
                                    stop=(k == n_kt - 1),
                                )
                        nc.vector.tensor_copy(whqT_sb, wq_ps)
                    # si row on the quarter [1, iw]
                    si_sb = prolog.tile([1, iw], f32, tag="si_sb")
                    with tc.tile_pool(name="ps_si", bufs=1,
                                      space="PSUM") as ps_si:
                        si_ps = ps_si.tile([1, iw], f32)
                        for c in range(iw // 512):
                            nc.tensor.matmul(
                                si_ps[:, c * 512 : (c + 1) * 512],
                                a12_sb[:, hp, 0:1],
                                whqT_sb[:, c * 512 : (c + 1) * 512],
                                start=True,
                                stop=True,
                            )
                        nc.vector.tensor_copy(si_sb, si_ps)
                    # Hi broadcast [128, iw]: exp(-0.8 si)
                    with tc.tile_pool(name="ps_sib", bufs=1,
                                      space="PSUM") as ps_sib:
                        sib_ps = ps_sib.tile([128, iw], f32)
                        for c in range(iw // 512):
                            nc.tensor.matmul(
                                sib_ps[:, c * 512 : (c + 1) * 512],
                                ones1,
                                si_sb[0:1, c * 512 : (c + 1) * 512],
                                start=True,
                                stop=True,
                            )
                        nc.scalar.activation(
                            Hi_bc[:, pp, :], sib_ps, AF.Exp, scale=-0.8
                        )
                    # Ej/Fj per-partition columns [128, n_jt]
                    with tc.tile_pool(name="ps_sj", bufs=1,
                                      space="PSUM") as ps_sj:
                        sj_ps = ps_sj.tile([128, n_jt], f32)
                        for t in range(n_jt):
                            nc.tensor.matmul(
                                sj_ps[:, t : t + 1],
                                whT_sb[:, t * 128 : (t + 1) * 128],
                                a12_sb[:, hp, 1:2],
                                start=True,
                                stop=True,
                            )
                        nc.scalar.activation(
                            Ej_cols[:, pp, :], sj_ps, AF.Exp
                        )
                        nc.scalar.activation(
                            Fj_cols[:, pp, :], sj_ps, AF.Exp,
                            scale=NEG_SLOPE,
                        )

            # --- main attention loop ---
            with (
                tc.tile_pool(name="work", bufs=8) as work,
                tc.tile_pool(name="fin", bufs=2) as fin,
                tc.tile_pool(name="ps_main", bufs=2, space="PSUM") as ps_main,
            ):
                for pp in [x for x in range(n_pair)] * repeat:
                    # 8 psum blocks of [128 i, 129(pad 256)] f32, two per
                    # 2KB bank; start=True clears has_written for the whole
                    # bank, so only the even block of a pair issues it.
                    out_ps = ps_main.tile([128, n_ib, 256], f32, tag="out_ps")
                    for jt in range(n_jt):
                        use_accum = (
                            accum_every > 0 and jt % accum_every == 0
                        )
                        m = work.tile([128, iw], bf16, tag="m")
                        if "dve" not in skip:
                            # P' pre-mask: max(Hi_i * Fj_j, Ej_j)
                            nc.vector.tensor_scalar(
                                m,
                                Hi_bc[:, pp, :],
                                Fj_cols[:, pp, jt : jt + 1],
                                Ej_cols[:, pp, jt : jt + 1],
                                ALU.mult,
                                ALU.max,
                            )
                        else:
                            nc.vector.memset(m, 1.0)
                        if use_accum:
                            if "dma" not in skip:
                                nc.gpsimd.dma_start(
                                    out=m,
                                    in_=(adjt_sb[:, jt, :] if accum_sbuf
                                         else adjt[jt * 128
                                                   : (jt + 1) * 128, :]),
                                    accum_op=ALU.add,
                                )
                            p = m
                            if "relu" not in skip:
                                nc.scalar.activation(p, p, AF.Relu)
                        else:
                            p = work.tile([128, iw], bf16, tag="p")
                            if "relu" not in skip:
                                nc.vector.tensor_mul(
                                    p, m, adjm_sb[:, jt, :]
                                )
                            else:
                                p = m
                        if "mm" in skip:
                            continue
                        for bi in range(n_ib):
                            nc.tensor.matmul(
                                out_ps[:, bi, 0 : d + 1],
                                p[:, bi * 128 : (bi + 1) * 128],
                                wh_ext[:, pp, jt, :],
                                start=(jt == 0 and bi % 2 == 0),
                                stop=(jt == n_jt - 1),
                                skip_group_check=(bi % 2 == 1),
                            )
                    # drain: one ACT copy for all 8 blocks, then DMA out
                    out_sb = fin.tile([128, n_ib, d + 1], f32, tag="out_sb")
                    if "mm" in skip:
                        nc.vector.memset(out_sb, 1.0)
                    else:
                        nc.scalar.activation(
                            out_sb, out_ps[:, :, 0 : d + 1], AF.Copy
                        )
                    nc.sync.dma_start(
                        out=outR[pp].rearrange("(b p) c -> p b c", p=128),
                        in_=out_sb,
                    )

    nc.compile()
    return nc


def _prep_inputs(h, adj, W, a):
    """Host-side shard/layout prep. Returns list of 8 per-core input dicts."""
    h_bf = np.ascontiguousarray(np.asarray(h).astype(BF16))
    adjT = np.asarray(adj).T != 0
    adjt_big = np.where(adjT, 0.0, MASK_VAL).astype(ml_dtypes.float8_e5m2)
    adjm_big = adjT.astype(BF16)
    W = np.asarray(W)
    a = np.asarray(a)
    in_maps = []
    for c in range(N_CORES):
        g, q = divmod(c, N_Q)
        isl = slice(q * IW, (q + 1) * IW)
        wt2 = np.ascontiguousarray(
            np.stack([W[2 * g].T, W[2 * g + 1].T])
        ).astype(BF16)                                    # [2, IN, D]
        a12 = np.ascontiguousarray(
            np.stack(
                [np.stack([a[2 * g + i, :D], a[2 * g + i, D:]], axis=1)
                 for i in range(2)]
            )
        ).astype(BF16)                                    # [2, D, 2]
        in_maps.append(
            {
                "hb": h_bf,
                "hq": np.ascontiguousarray(h_bf[:, isl, :]),
                "wt2": wt2,
                "a12": a12,
                "adjt": np.ascontiguousarray(adjt_big[:, isl]),
                "adjm": np.ascontiguousarray(adjm_big[:, isl]),
            }
        )
    return in_maps


def kernel(h, adj, W, a):
    from concourse.bass_utils import run_bass_kernel_spmd

    if "nc" not in _cache:
        _cache["nc"] = _build()
    nc = _cache["nc"]

    in_maps = _prep_inputs(h, adj, W, a)
    res = run_bass_kernel_spmd(nc, in_maps, core_ids=list(range(N_CORES)))
    outs = [r["outR"] for r in res.results]  # each [4, IW, D+1] f32

    out = np.zeros((B, N, D), dtype=np.float32)
    for c in range(N_CORES):
        g, q = divmod(c, N_Q)
        isl = slice(q * IW, (q + 1) * IW)
        for pp in range(4):
            hp, bp = divmod(pp, B)
            o = outs[c][pp]
            r = o[:, D:]
            out[bp, isl] += np.divide(
                o[:, :D], r, out=np.zeros((IW, D), np.float32),
                where=r != 0,
            )
    return out
